# revision 5
# baseline (speedup 1.0000x reference)
"""Trainium2 Bass kernel for nn_CorrelationFilter (SiamFC-style correlation).

Math (per batch pair b):
    out[b, oi, oj] = sum_{di<6, dj<6, c<256} x[b, oi+di, oj+dj, c] * z[b, di, dj, c]
                     + sum_{c<256} bias[0, oi, oj, b*256 + c]
with x: [B,22,22,256], z: [B,6,6,256], bias: [1,17,17,B*256], out: [B,17,17,1].

Strategy: pure data parallelism over batch across 8 NeuronCores (16 batches per
core), no cross-core communication. Host does sharding + layout prep only
(transpose to channel-major, cast to bf16); all arithmetic runs on device.

Per core, DM = DI_MERGE, NK = 6/DM di-blocks, G = DM*6 fold groups. Batches are
packed RS-row groups into PSUM banks (QB = 128/RS batches per bank):
  - Q matmuls (per batch, per ch, per k): stationary zT[:, b, ch, k, :] (K=128,
    M=RS, cols >= G zero), moving xT[ch][:, b, 22*DM*k : +NMOV], accumulating
    at rows RS*bb of the bank:
      q[RS*bb + g, m] = sum_{ch,c,k} z[b, DM*k+dd, dj, c] * x[b, c, 22*DM*k+m]
  - bias matmuls: stationary = ones column at col G -> row RS*bb+G gets
    sum_c bias[o, b, c] over both ch; rows < G get +0.
  - 1 ScalarE evacuation per bank: [128, NMOV] PSUM -> SBUF fp16
  - bounce through DRAM (SBUF-side DMA APs must be plain partition ranges);
    the gather back applies the per-group shift via affine AP strides:
      t_sh[b, g, m'] = q_b[g, m' + 22*(g//6) + g%6]
  - fold: ONE VectorE tensor_reduce over the g axis of a strided 4-dim view
      acc[b, oi, oj] = sum_g t_sh[b, g, 22*oi + oj]
  - final add of the (dense-289) bias row + output DMA, batches in natural
    order (b = QB*bank + bb), no permutation anywhere.

kernel(**inputs) takes FULL unsharded inputs, returns the full output.
"""

import os
import numpy as np
import ml_dtypes

import concourse.bass as bass
import concourse.mybir as mybir
from concourse import bacc
from concourse.tile import TileContext

B, H, W, C = 128, 22, 22, 256
HZ, WZ = 6, 6
HO, WO = 17, 17
OO = HO * WO               # 289 dense output positions
NCORES = 8
BPC = B // NCORES          # 16 batches per core
P = H * W                  # 484 flattened search positions
O22 = (HO - 1) * W + WO    # 369: output span in 22-wide layout
OPAD = HO * W              # 374: padded span so m' factors as (oi, 22)

DI_MERGE = int(os.environ.get("KERNEL_DI_MERGE", "3"))
NK = HZ // DI_MERGE                      # matmul k-blocks per (ch)
G = DI_MERGE * WZ                        # fold groups per batch
NMOV = O22 + (DI_MERGE - 1) * W + (WZ - 1)  # moving cols per Q matmul
RS = 32 if G + 1 <= 32 else 64           # PSUM row stride per batch
QB = 128 // RS                           # batches per PSUM bank
NBANK = BPC // QB

_BF16 = mybir.dt.bfloat16
_F16 = mybir.dt.float16
_F32 = mybir.dt.float32


def build_module():
    assert G + 1 <= RS and NMOV * 4 <= 2048
    nc = bacc.Bacc()
    xt_d = nc.dram_tensor("xt", [2, 128, BPC, P], _BF16, kind="ExternalInput")
    zt_d = nc.dram_tensor("zt", [128, BPC, 2, NK, RS], _BF16, kind="ExternalInput")
    bt_d = nc.dram_tensor("bt", [2, 128, BPC, OO], _BF16, kind="ExternalInput")
    out_d = nc.dram_tensor("out", [BPC, HO, WO], _F32, kind="ExternalOutput")

    with TileContext(nc) as tc:
        with (
            tc.tile_pool(name="const", bufs=1) as cpool,
            tc.tile_pool(name="big", bufs=1) as big,
            tc.tile_pool(name="work", bufs=2) as work,
            tc.tile_pool(name="psum", bufs=NBANK, space="PSUM") as psum,
            tc.tile_pool(name="dram", bufs=1, space="DRAM") as dpool,
        ):
            # ones column at col G, zeros elsewhere: bias-row stationary
            onesp = cpool.tile([128, RS], _BF16, name="onesp")
            nc.gpsimd.memset(onesp[:], 0.0)
            nc.gpsimd.memset(onesp[:, G : G + 1], 1.0)

            # stationary z: host-prepped channel-major, contiguous DMA
            zt_t = cpool.tile([128, BPC, 2, NK, RS], _BF16, name="ztt")
            nc.scalar.dma_start(out=zt_t[:], in_=zt_d[:])

            bt_t = [
                big.tile([128, BPC, OO], _BF16, name=f"bt{ch}", tag=f"bt{ch}")
                for ch in range(2)
            ]
            for ch in range(2):
                for b0 in range(0, BPC, 8):
                    nc.scalar.dma_start(
                        out=bt_t[ch][:, b0 : b0 + 8, :],
                        in_=bt_d[ch, :, b0 : b0 + 8, :],
                    )

            # moving x, dispatched in growing chunks so batch 0 lands ASAP;
            # all dispatches up front so gathers (later on Sync) can't delay
            # them.
            xt_t = [
                big.tile([128, BPC, P], _BF16, name=f"xt{ch}", tag=f"xt{ch}")
                for ch in range(2)
            ]
            chunks = [(0, 1), (1, 1), (2, 2), (4, 4), (8, 8)]
            for b0, n in chunks:
                for ch in range(2):
                    nc.sync.dma_start(
                        out=xt_t[ch][:, b0 : b0 + n, :],
                        in_=xt_d[ch, :, b0 : b0 + n, :],
                    )

            # fold input, batch-major: t_sh[b, g, m] = Q_b[g, m]; row G is the
            # dense-289 bias row
            t_sh = big.tile([BPC, G + 1, NMOV], _F16, name="tsh")

            scrs = [
                dpool.tile([128, NMOV], _F16, name=f"scr{k}", tag=f"scr{k}")
                for k in range(NBANK)
            ]
            for k in range(NBANK):
                q = psum.tile([128, NMOV], _F32, name="q", tag="q", bufs=NBANK)
                for bb in range(QB):
                    b = QB * k + bb
                    qmms = [(ch, kk) for ch in range(2) for kk in range(NK)]
                    for i, (ch, kk) in enumerate(qmms):
                        s = 22 * DI_MERGE * kk
                        nc.tensor.matmul(
                            q[RS * bb : RS * bb + RS, :],
                            zt_t[:, b, ch, kk, :],
                            xt_t[ch][:, b, s : s + NMOV],
                            start=(i == 0),
                            stop=(i == len(qmms) - 1),
                            tile_position=(0, RS * bb),
                        )
                        if i == 0:
                            for ch2 in range(2):
                                nc.tensor.matmul(
                                    q[RS * bb : RS * bb + RS, 0:OO],
                                    onesp[:],
                                    bt_t[ch2][:, b, :],
                                    start=False,
                                    stop=False,
                                    tile_position=(0, RS * bb),
                                )
                # one evacuation per bank, downcast to fp16
                e = work.tile([128, NMOV], _F16, name="e", tag="e")
                nc.scalar.copy(out=e[:], in_=q[:])
                nc.scalar.dma_start(out=scrs[k][:], in_=e[:])
                # gather rows (g, bias) of each RS-quadrant into batch rows
                sflat = scrs[k][:].rearrange("p m -> (p m)")
                nc.sync.dma_start(
                    out=t_sh[QB * k : QB * k + QB, :, :],
                    in_=bass.AP(
                        sflat.tensor,
                        sflat.offset,
                        [[RS * NMOV, QB], [NMOV, G + 1], [1, NMOV]],
                    ),
                )

            # fold: acc[b, oi, oj] = sum_{dd, dj} t_sh[b, 6*dd+dj,
            # 22*(oi+dd) + oj+dj] — one DVE reduce over a 5-dim AP whose
            # (dd, dj) strides carry the shift coupling
            acc = work.tile([BPC, HO, WO], _F32, name="acc")
            tv = t_sh[:, :, :]
            pitch = tv.ap[0][0]
            fold_in = bass.AP(
                tv.tensor,
                tv.offset,
                [
                    [pitch, BPC],
                    [W, HO],
                    [1, WO],
                    [WZ * NMOV + W, DI_MERGE],
                    [NMOV + 1, WZ],
                ],
            )
            nc.vector.tensor_reduce(
                out=acc[:], in_=fold_in, axis=mybir.AxisListType.XY,
                op=mybir.AluOpType.add,
            )
            outb = work.tile([BPC, HO, WO], _F32, name="outb")
            bias_in = bass.AP(
                tv.tensor,
                tv.offset + G * NMOV,
                [[pitch, BPC], [WO, HO], [1, WO]],
            )
            nc.vector.tensor_tensor(
                out=outb[:],
                in0=acc[:],
                in1=bias_in,
                op=mybir.AluOpType.add,
            )
            nc.scalar.dma_start(out=out_d[:], in_=outb[:])

    nc.compile()
    return nc


def prep_inputs(x, z, b):
    """Host-side shard + layout prep. Returns per-core in_maps."""
    xb = np.asarray(x).astype(ml_dtypes.bfloat16)
    zb = np.asarray(z).astype(ml_dtypes.bfloat16)
    bb = np.asarray(b).astype(ml_dtypes.bfloat16)
    bias3 = bb.reshape(OO, B, C)
    in_maps = []
    for core in range(NCORES):
        b0 = core * BPC
        xs = xb[b0 : b0 + BPC].reshape(BPC, P, C)
        xT = np.ascontiguousarray(xs.transpose(2, 0, 1)).reshape(2, 128, BPC, P)
        # zT[c, b, ch, k, g] = z[b, DM*k + g//6, g%6, ch*128 + c]
        zs = zb[b0 : b0 + BPC].reshape(BPC, NK, G, C)
        zT = np.zeros((128, BPC, 2, NK, RS), dtype=ml_dtypes.bfloat16)
        zT[..., :G] = (
            np.ascontiguousarray(zs.transpose(3, 0, 1, 2))
            .reshape(2, 128, BPC, NK, G)
            .transpose(1, 2, 0, 3, 4)
        )
        bs = bias3[:, b0 : b0 + BPC, :]
        bT = np.ascontiguousarray(bs.transpose(2, 1, 0)).reshape(2, 128, BPC, OO)
        in_maps.append({"xt": xT, "zt": zT, "bt": bT})
    return in_maps


_cache = {}


def _ensure_ntff_hook():
    """The axon NTFF profile hook normally lives in antenv.axon_hooks, which
    this image lacks; synthesize it from the boot shim's ctypes wrapper."""
    try:
        from antenv.axon_hooks import get_axon_ntff_profile_hook  # noqa: F401
        return True
    except ImportError:
        pass
    try:
        import sys, types
        from trn_agent_boot.trn_boot import _ntff_profile_via_ctypes

        so = os.environ.get("AXON_PJRT_SO", "/opt/axon/libaxon_pjrt.so")
        hook = _ntff_profile_via_ctypes(so)
        mod = types.ModuleType("antenv.axon_hooks")
        mod.get_axon_ntff_profile_hook = lambda: hook
        mod.set_axon_ntff_profile_hook = lambda h: None
        sys.modules["antenv.axon_hooks"] = mod
        import antenv

        antenv.axon_hooks = mod
        return True
    except Exception:
        return False


def kernel(x, z, b):
    from concourse.bass_utils import run_bass_kernel_spmd

    if "nc" not in _cache:
        _cache["nc"] = build_module()
    nc = _cache["nc"]
    in_maps = prep_inputs(x, z, b)
    trace = bool(int(os.environ.get("KERNEL_TRACE", "0") or 0))
    if trace:
        trace = _ensure_ntff_hook()
    res = run_bass_kernel_spmd(
        nc,
        in_maps,
        core_ids=list(range(NCORES)),
        trace=trace,
    )
    _cache["last_result"] = res
    out = np.concatenate([r["out"].reshape(BPC, HO, WO) for r in res.results], axis=0)
    return out[..., None].astype(np.float32)


# revision 13
# speedup vs baseline: 1.0713x; 1.0713x over previous
"""Trainium2 Bass kernel for nn_CorrelationFilter (SiamFC-style correlation).

Math (per batch pair b):
    out[b, oi, oj] = sum_{di<6, dj<6, c<256} x[b, oi+di, oj+dj, c] * z[b, di, dj, c]
                     + sum_{c<256} bias[0, oi, oj, b*256 + c]
with x: [B,22,22,256], z: [B,6,6,256], bias: [1,17,17,B*256], out: [B,17,17,1].

Strategy: pure data parallelism over batch across 8 NeuronCores (16 batches per
core), no cross-core communication. Host does sharding + layout prep only
(transpose to channel-major, cast to bf16); all arithmetic runs on device.

Per core, DM = DI_MERGE, NK = 6/DM di-blocks, G = DM*6 fold groups. Batches are
packed RS-row groups into PSUM banks (QB = 128/RS batches per bank):
  - Q matmuls (per batch, per ch, per k): stationary zT[:, b, ch, k, :] (K=128,
    M=RS, cols >= G zero), moving xT[ch][:, b, 22*DM*k : +NMOV], accumulating
    at rows RS*bb of the bank:
      q[RS*bb + g, m] = sum_{ch,c,k} z[b, DM*k+dd, dj, c] * x[b, c, 22*DM*k+m]
  - bias matmuls: stationary = ones column at col G -> row RS*bb+G gets
    sum_c bias[o, b, c] over both ch; rows < G get +0.
  - 1 ScalarE evacuation per bank: [128, NMOV] PSUM -> SBUF fp16
  - bounce through DRAM (SBUF-side DMA APs must be plain partition ranges);
    the gather back applies the per-group shift via affine AP strides:
      t_sh[b, g, m'] = q_b[g, m' + 22*(g//6) + g%6]
  - fold: ONE VectorE tensor_reduce over the g axis of a strided 4-dim view
      acc[b, oi, oj] = sum_g t_sh[b, g, 22*oi + oj]
  - final add of the (dense-289) bias row + output DMA, batches in natural
    order (b = QB*bank + bb), no permutation anywhere.

kernel(**inputs) takes FULL unsharded inputs, returns the full output.
"""

import os
import numpy as np
import ml_dtypes

import concourse.bass as bass
import concourse.mybir as mybir
from concourse import bacc
from concourse.tile import TileContext

B, H, W, C = 128, 22, 22, 256
HZ, WZ = 6, 6
HO, WO = 17, 17
OO = HO * WO               # 289 dense output positions
NCORES = 8
BPC = B // NCORES          # 16 batches per core
P = H * W                  # 484 flattened search positions
O22 = (HO - 1) * W + WO    # 369: output span in 22-wide layout
OPAD = HO * W              # 374: padded span so m' factors as (oi, 22)

DI_MERGE = int(os.environ.get("KERNEL_DI_MERGE", "3"))
NK = HZ // DI_MERGE                      # matmul k-blocks per (ch)
G = DI_MERGE * WZ                        # fold groups per batch
NMOV = O22 + (DI_MERGE - 1) * W + (WZ - 1)  # moving cols per Q matmul
RS = 32 if G + 1 <= 32 else 64           # PSUM row stride per batch
QB = 128 // RS                           # batches per PSUM bank
NBANK = BPC // QB

# fold chunking: DVE time scales with free-dim size only, so pack
# (chunk, batch) into partitions to cut the per-partition free size
NCHUNK = int(os.environ.get("KERNEL_NCHUNK", "4"))
OIB = (HO + NCHUNK - 1) // NCHUNK        # output rows per chunk
FLM = W * (OIB - 1) + WO + W * (DI_MERGE - 1) + (WZ - 1)  # chunk read extent
EVDT_NAME = os.environ.get("KERNEL_EVDT", "fp16")

_BF16 = mybir.dt.bfloat16
_F16 = mybir.dt.float16
_F32 = mybir.dt.float32


def build_module():
    assert G + 1 <= RS and NMOV * 4 <= 2048
    nc = bacc.Bacc()
    xt_d = nc.dram_tensor("xt", [2, 128, BPC, P], _BF16, kind="ExternalInput")
    zt_d = nc.dram_tensor("zt", [128, BPC, 2, NK, RS], _BF16, kind="ExternalInput")
    bt_d = nc.dram_tensor("bt", [2, 128, BPC, OO], _BF16, kind="ExternalInput")
    out_d = nc.dram_tensor("out", [BPC, HO, WO], _F32, kind="ExternalOutput")

    with TileContext(nc) as tc:
        with (
            tc.tile_pool(name="const", bufs=1) as cpool,
            tc.tile_pool(name="big", bufs=1) as big,
            tc.tile_pool(name="work", bufs=2) as work,
            tc.tile_pool(name="psum", bufs=NBANK, space="PSUM") as psum,
            tc.tile_pool(name="dram", bufs=1, space="DRAM") as dpool,
        ):
            # ones column at col G, zeros elsewhere: bias-row stationary
            onesp = cpool.tile([128, RS], _BF16, name="onesp")
            nc.gpsimd.memset(onesp[:], 0.0)
            nc.gpsimd.memset(onesp[:, G : G + 1], 1.0)

            # stationary z: host-prepped channel-major, contiguous DMA
            zt_t = cpool.tile([128, BPC, 2, NK, RS], _BF16, name="ztt")
            nc.scalar.dma_start(out=zt_t[:], in_=zt_d[:])

            bt_t = [
                big.tile([128, BPC, OO], _BF16, name=f"bt{ch}", tag=f"bt{ch}")
                for ch in range(2)
            ]
            for ch in range(2):
                for b0 in range(0, BPC, 8):
                    nc.scalar.dma_start(
                        out=bt_t[ch][:, b0 : b0 + 8, :],
                        in_=bt_d[ch, :, b0 : b0 + 8, :],
                    )

            # moving x, dispatched in growing chunks so batch 0 lands ASAP;
            # all dispatches up front so gathers (later on Sync) can't delay
            # them.
            xt_t = [
                big.tile([128, BPC, P], _BF16, name=f"xt{ch}", tag=f"xt{ch}")
                for ch in range(2)
            ]
            chunks = [(0, 1), (1, 1), (2, 2), (4, 4), (8, 8)]
            for b0, n in chunks:
                for ch in range(2):
                    nc.sync.dma_start(
                        out=xt_t[ch][:, b0 : b0 + n, :],
                        in_=xt_d[ch, :, b0 : b0 + n, :],
                    )

            EVDT = _F16 if EVDT_NAME == "fp16" else _F32
            # fold input, one 32-partition block per chunk (batches in the
            # lower 16 rows; junk rows ride along in lockstep for free):
            #   fl[32*c + b, g, mm] = Q_b[g, W*OIB*c + mm]
            fl = big.tile([32 * NCHUNK, G, FLM], EVDT, name="fl")
            # dense-289 bias rows: tb[b, o] = sum_c bias[o, b, c]
            tb = big.tile([BPC, OO], EVDT, name="tb")
            # keep the never-written rows/cols the lockstep fold reads finite
            nc.gpsimd.memset(fl[:], 0.0)

            scrs = [
                dpool.tile([128, NMOV], EVDT, name=f"scr{k}", tag=f"scr{k}")
                for k in range(NBANK)
            ]
            for k in range(NBANK):
                q = psum.tile([128, NMOV], _F32, name="q", tag="q", bufs=2)
                for bb in range(QB):
                    b = QB * k + bb
                    qmms = [(ch, kk) for ch in range(2) for kk in range(NK)]
                    for i, (ch, kk) in enumerate(qmms):
                        s = 22 * DI_MERGE * kk
                        nc.tensor.matmul(
                            q[RS * bb : RS * bb + RS, :],
                            zt_t[:, b, ch, kk, :],
                            xt_t[ch][:, b, s : s + NMOV],
                            start=(i == 0),
                            stop=(i == len(qmms) - 1),
                            tile_position=(0, RS * bb),
                        )
                        if i == 0:
                            for ch2 in range(2):
                                nc.tensor.matmul(
                                    q[RS * bb : RS * bb + RS, 0:OO],
                                    onesp[:],
                                    bt_t[ch2][:, b, :],
                                    start=False,
                                    stop=False,
                                    tile_position=(0, RS * bb),
                                )
                # one evacuation per bank, downcast
                e = work.tile([128, NMOV], EVDT, name="e", tag="e")
                nc.scalar.copy(out=e[:], in_=q[:])
                nc.scalar.dma_start(out=scrs[k][:], in_=e[:])
                sflat = scrs[k][:].rearrange("p m -> (p m)")
                # bias rows (r = G of each RS-quadrant), dense 289
                nc.scalar.dma_start(
                    out=tb[QB * k : QB * k + QB, :],
                    in_=bass.AP(
                        sflat.tensor,
                        sflat.offset + G * NMOV,
                        [[RS * NMOV, QB], [1, OO]],
                    ),
                )
                # per-chunk gathers of the g rows into (chunk, batch) rows
                for c in range(NCHUNK):
                    m0 = W * OIB * c
                    wc = min(FLM, NMOV - m0)
                    eng = nc.sync if c % 2 == 0 else nc.scalar
                    eng.dma_start(
                        out=fl[c * 32 + QB * k : c * 32 + QB * k + QB, :, 0:wc],
                        in_=bass.AP(
                            sflat.tensor,
                            sflat.offset + m0,
                            [[RS * NMOV, QB], [NMOV, G], [1, wc]],
                        ),
                    )

            # fold: acc[16c + b, oi', oj] = sum_{dd, dj} fl[16c + b, 6*dd+dj,
            # 22*(oi'+dd) + oj+dj] — one DVE reduce over a 5-dim AP whose
            # (dd, dj) strides carry the shift coupling
            acc = work.tile([32 * NCHUNK, OIB, WO], _F32, name="acc")
            tv = fl[:, :, :]
            pitch = tv.ap[0][0]
            fold_in = bass.AP(
                tv.tensor,
                tv.offset,
                [
                    [pitch, 32 * NCHUNK],
                    [W, OIB],
                    [1, WO],
                    [WZ * FLM + W, DI_MERGE],
                    [FLM + 1, WZ],
                ],
            )
            nc.vector.tensor_reduce(
                out=acc[:], in_=fold_in, axis=mybir.AxisListType.XY,
                op=mybir.AluOpType.add,
            )
            # assemble chunks (single-input copies may shift partitions),
            # then add the bias rows in place
            outb = work.tile([BPC, HO, WO], _F32, name="outb")
            for c in range(NCHUNK):
                n = min(OIB, HO - OIB * c)
                nc.gpsimd.tensor_copy(
                    outb[:, OIB * c : OIB * c + n, :],
                    acc[c * 32 : c * 32 + BPC, 0:n, :],
                )
            nc.vector.tensor_tensor(
                out=outb[:],
                in0=outb[:],
                in1=tb[:].rearrange("b (i j) -> b i j", j=WO),
                op=mybir.AluOpType.add,
            )
            nc.scalar.dma_start(out=out_d[:], in_=outb[:])

    nc.compile()
    return nc


def prep_inputs(x, z, b):
    """Host-side shard + layout prep. Returns per-core in_maps."""
    xb = np.asarray(x).astype(ml_dtypes.bfloat16)
    zb = np.asarray(z).astype(ml_dtypes.bfloat16)
    bb = np.asarray(b).astype(ml_dtypes.bfloat16)
    bias3 = bb.reshape(OO, B, C)
    in_maps = []
    for core in range(NCORES):
        b0 = core * BPC
        xs = xb[b0 : b0 + BPC].reshape(BPC, P, C)
        xT = np.ascontiguousarray(xs.transpose(2, 0, 1)).reshape(2, 128, BPC, P)
        # zT[c, b, ch, k, g] = z[b, DM*k + g//6, g%6, ch*128 + c]
        zs = zb[b0 : b0 + BPC].reshape(BPC, NK, G, C)
        zT = np.zeros((128, BPC, 2, NK, RS), dtype=ml_dtypes.bfloat16)
        zT[..., :G] = (
            np.ascontiguousarray(zs.transpose(3, 0, 1, 2))
            .reshape(2, 128, BPC, NK, G)
            .transpose(1, 2, 0, 3, 4)
        )
        bs = bias3[:, b0 : b0 + BPC, :]
        bT = np.ascontiguousarray(bs.transpose(2, 1, 0)).reshape(2, 128, BPC, OO)
        in_maps.append({"xt": xT, "zt": zT, "bt": bT})
    return in_maps


_cache = {}


def _ensure_ntff_hook():
    """The axon NTFF profile hook normally lives in antenv.axon_hooks, which
    this image lacks; synthesize it from the boot shim's ctypes wrapper."""
    try:
        from antenv.axon_hooks import get_axon_ntff_profile_hook  # noqa: F401
        return True
    except ImportError:
        pass
    try:
        import sys, types
        from trn_agent_boot.trn_boot import _ntff_profile_via_ctypes

        so = os.environ.get("AXON_PJRT_SO", "/opt/axon/libaxon_pjrt.so")
        hook = _ntff_profile_via_ctypes(so)
        mod = types.ModuleType("antenv.axon_hooks")
        mod.get_axon_ntff_profile_hook = lambda: hook
        mod.set_axon_ntff_profile_hook = lambda h: None
        sys.modules["antenv.axon_hooks"] = mod
        import antenv

        antenv.axon_hooks = mod
        return True
    except Exception:
        return False


def kernel(x, z, b):
    from concourse.bass_utils import run_bass_kernel_spmd

    if "nc" not in _cache:
        _cache["nc"] = build_module()
    nc = _cache["nc"]
    in_maps = prep_inputs(x, z, b)
    trace = bool(int(os.environ.get("KERNEL_TRACE", "0") or 0))
    if trace:
        trace = _ensure_ntff_hook()
    res = run_bass_kernel_spmd(
        nc,
        in_maps,
        core_ids=list(range(NCORES)),
        trace=trace,
    )
    _cache["last_result"] = res
    out = np.concatenate([r["out"].reshape(BPC, HO, WO) for r in res.results], axis=0)
    return out[..., None].astype(np.float32)


# revision 14
# speedup vs baseline: 1.2303x; 1.1484x over previous
"""Trainium2 Bass kernel for nn_CorrelationFilter (SiamFC-style correlation).

Math (per batch pair b):
    out[b, oi, oj] = sum_{di<6, dj<6, c<256} x[b, oi+di, oj+dj, c] * z[b, di, dj, c]
                     + sum_{c<256} bias[0, oi, oj, b*256 + c]
with x: [B,22,22,256], z: [B,6,6,256], bias: [1,17,17,B*256], out: [B,17,17,1].

Strategy: pure data parallelism over batch across 8 NeuronCores (16 batches per
core), no cross-core communication. Host does sharding + layout prep only
(transpose to channel-major, cast to bf16); all arithmetic runs on device.

Per core, DM = DI_MERGE, NK = 6/DM di-blocks, G = DM*6 fold groups. Batches are
packed RS rows apart into PSUM banks (QB = 128/RS batches per bank):
  - Q matmuls (per batch, ch, k): stationary zT[:, b, ch, k, :] (K=128, M=ZC),
    moving xT[:, ch, b, 22*DM*k : +NMOV], accumulating at rows RS*bb:
      q[RS*bb + g, m] = sum_{ch,c,k} z[b, DM*k+dd, dj, c] * x[b, c, 22*DM*k+m]
  - bias matmuls: stationary = ones column at col G -> row RS*bb+G gets
    sum_c bias[o, b, c] over both ch; rows < G get +0.
  - 1 VectorE evacuation per bank: [128, NMOV] PSUM -> SBUF fp16
  - bounce through DRAM (SBUF-side DMA APs must be plain partition ranges),
    two banks per scratch tensor so one gather covers 8 batches;
    per-chunk gathers land (chunk-of-rows, batch) in 32-partition blocks:
      fl[32*c + b, g, mm] = Q_b[g, 22*OIB*c + mm]
  - fold: ONE VectorE tensor_reduce over a 5-dim AP whose (dd, dj) strides
    carry the shift coupling; DVE time scales with free size only, so the
    chunking packs 128 partitions to cut it 4x:
      acc[32c + b, oi', oj] = sum_{dd,dj} fl[32c+b, 6dd+dj, 22(oi'+dd)+oj+dj]
  - assembly copies + one bias add + output DMA, batches in natural order.

DMA dispatch on the Sync/Act queue engines costs ~0.7us per dma_start, so the
kernel merges tensors and gathers aggressively to keep the dispatch count low.

kernel(**inputs) takes FULL unsharded inputs, returns the full output.
"""

import os
import numpy as np
import ml_dtypes

import concourse.bass as bass
import concourse.mybir as mybir
from concourse import bacc
from concourse.tile import TileContext

B, H, W, C = 128, 22, 22, 256
HZ, WZ = 6, 6
HO, WO = 17, 17
OO = HO * WO               # 289 dense output positions
NCORES = 8
BPC = B // NCORES          # 16 batches per core
P = H * W                  # 484 flattened search positions
O22 = (HO - 1) * W + WO    # 369: output span in 22-wide layout

DI_MERGE = int(os.environ.get("KERNEL_DI_MERGE", "3"))
NK = HZ // DI_MERGE                      # matmul k-blocks per (ch)
G = DI_MERGE * WZ                        # fold groups per batch
NMOV = O22 + (DI_MERGE - 1) * W + (WZ - 1)  # moving cols per Q matmul
RS = 32 if G + 1 <= 32 else 64           # PSUM row stride per batch
QB = 128 // RS                           # batches per PSUM bank
NBANK = BPC // QB
ZC = G + 2                               # stationary cols (G z-cols, bias, pad)

NCHUNK = int(os.environ.get("KERNEL_NCHUNK", "4"))
OIB = (HO + NCHUNK - 1) // NCHUNK        # output rows per chunk
FLM = W * (OIB - 1) + WO + W * (DI_MERGE - 1) + (WZ - 1)  # chunk read extent
EVDT_NAME = os.environ.get("KERNEL_EVDT", "fp16")

_BF16 = mybir.dt.bfloat16
_F16 = mybir.dt.float16
_F32 = mybir.dt.float32


def build_module():
    assert G + 1 <= RS and NMOV * 4 <= 2048
    nc = bacc.Bacc()
    xt_d = nc.dram_tensor("xt", [128, 2, BPC, P], _BF16, kind="ExternalInput")
    zt_d = nc.dram_tensor("zt", [128, BPC, 2, NK, ZC], _BF16, kind="ExternalInput")
    bt_d = nc.dram_tensor("bt", [128, 2, BPC, OO], _BF16, kind="ExternalInput")
    out_d = nc.dram_tensor("out", [BPC, HO, WO], _F32, kind="ExternalOutput")

    with TileContext(nc) as tc:
        with (
            tc.tile_pool(name="const", bufs=1) as cpool,
            tc.tile_pool(name="big", bufs=1) as big,
            tc.tile_pool(name="work", bufs=2) as work,
            tc.tile_pool(name="psum", bufs=2, space="PSUM") as psum,
            tc.tile_pool(name="dram", bufs=1, space="DRAM") as dpool,
        ):
            # ones column at col G, zeros elsewhere: bias-row stationary
            onesp = cpool.tile([128, ZC], _BF16, name="onesp")
            nc.gpsimd.memset(onesp[:], 0.0)
            nc.gpsimd.memset(onesp[:, G : G + 1], 1.0)

            # stationary z, host-prepped channel-major; batch 0-1 slice first
            # so the first matmuls are not gated on the full transfer
            zt_t = cpool.tile([128, BPC, 2, NK, ZC], _BF16, name="ztt")
            nc.scalar.dma_start(out=zt_t[:, 0:2], in_=zt_d[:, 0:2])
            nc.scalar.dma_start(out=zt_t[:, 2:BPC], in_=zt_d[:, 2:BPC])

            bt_t = big.tile([128, 2, BPC, OO], _BF16, name="btt")
            nc.scalar.dma_start(out=bt_t[:], in_=bt_d[:])

            # moving x, growing chunks so batch 0 lands ASAP
            xt_t = big.tile([128, 2, BPC, P], _BF16, name="xtt")
            for b0, n in ((0, 1), (1, 1), (2, 2), (4, 4), (8, 8)):
                nc.sync.dma_start(
                    out=xt_t[:, :, b0 : b0 + n, :], in_=xt_d[:, :, b0 : b0 + n, :]
                )

            EVDT = _F16 if EVDT_NAME == "fp16" else _F32
            # fold input, one 32-partition block per chunk (batches of both
            # bank-pairs in the lower 16 rows; junk rows ride along in
            # lockstep for free): fl[32*c + b, g, mm] = Q_b[g, W*OIB*c + mm]
            fl = big.tile([32 * NCHUNK, G, FLM], EVDT, name="fl")
            # dense-289 bias rows: tb[b, o] = sum_c bias[o, b, c]
            tb = big.tile([BPC, OO], EVDT, name="tb")
            # keep the never-written rows/cols the lockstep fold reads finite
            nc.gpsimd.memset(fl[:], 0.0)

            # two banks per DRAM scratch so one gather covers 8 batches
            scrp = [
                dpool.tile([2, 128, NMOV], EVDT, name=f"scrp{p}", tag=f"scrp{p}")
                for p in range(NBANK // 2)
            ]
            for k in range(NBANK):
                q = psum.tile([128, NMOV], _F32, name="q", tag="q", bufs=2)
                for bb in range(QB):
                    b = QB * k + bb
                    qmms = [(ch, kk) for ch in range(2) for kk in range(NK)]
                    for i, (ch, kk) in enumerate(qmms):
                        s = 22 * DI_MERGE * kk
                        nc.tensor.matmul(
                            q[RS * bb : RS * bb + ZC, :],
                            zt_t[:, b, ch, kk, :],
                            xt_t[:, ch, b, s : s + NMOV],
                            start=(i == 0),
                            stop=(i == len(qmms) - 1),
                            tile_position=(0, RS * bb),
                        )
                        if i == 0:
                            for ch2 in range(2):
                                nc.tensor.matmul(
                                    q[RS * bb : RS * bb + ZC, 0:OO],
                                    onesp[:],
                                    bt_t[:, ch2, b, :],
                                    start=False,
                                    stop=False,
                                    tile_position=(0, RS * bb),
                                )
                # one evacuation per bank (VectorE: scalar would need its
                # activation table; vector is idle mid-window anyway)
                e = work.tile([128, NMOV], EVDT, name="e", tag="e")
                nc.vector.tensor_copy(e[:], q[:])
                nc.scalar.dma_start(out=scrp[k // 2][k % 2], in_=e[:])

                if k % 2 == 1:
                    p = k // 2
                    sv = scrp[p][:]
                    # pair gathers: src (q, bb, g, mm) with the (q, bb) dims
                    # merged (q stride = 128*NMOV = QB*RS*NMOV)
                    for c in range(NCHUNK):
                        m0 = W * OIB * c
                        wc = min(FLM, NMOV - m0)
                        eng = nc.sync if c % 2 == 0 else nc.scalar
                        eng.dma_start(
                            out=fl[c * 32 + 8 * p : c * 32 + 8 * p + 8, :, 0:wc],
                            in_=bass.AP(
                                sv.tensor,
                                sv.offset + m0,
                                [[RS * NMOV, 2 * QB], [NMOV, G], [1, wc]],
                            ),
                        )
                    # bias rows (r = G of each RS-quadrant), dense 289
                    nc.scalar.dma_start(
                        out=tb[8 * p : 8 * p + 8, :],
                        in_=bass.AP(
                            sv.tensor,
                            sv.offset + G * NMOV,
                            [[RS * NMOV, 2 * QB], [1, OO]],
                        ),
                    )

            # fold: acc[32c + b, oi', oj] = sum_{dd, dj} fl[32c + b, 6*dd+dj,
            # 22*(oi'+dd) + oj+dj] — one DVE reduce over a 5-dim AP whose
            # (dd, dj) strides carry the shift coupling
            acc = work.tile([32 * NCHUNK, OIB, WO], _F32, name="acc")
            tv = fl[:, :, :]
            pitch = tv.ap[0][0]
            fold_in = bass.AP(
                tv.tensor,
                tv.offset,
                [
                    [pitch, 32 * NCHUNK],
                    [W, OIB],
                    [1, WO],
                    [WZ * FLM + W, DI_MERGE],
                    [FLM + 1, WZ],
                ],
            )
            nc.vector.tensor_reduce(
                out=acc[:], in_=fold_in, axis=mybir.AxisListType.XY,
                op=mybir.AluOpType.add,
            )
            # assemble chunks (single-input copies may shift partitions),
            # then add the bias rows in place
            outb = work.tile([BPC, HO, WO], _F32, name="outb")
            for c in range(NCHUNK):
                n = min(OIB, HO - OIB * c)
                nc.vector.tensor_copy(
                    outb[:, OIB * c : OIB * c + n, :],
                    acc[c * 32 : c * 32 + BPC, 0:n, :],
                )
            nc.vector.tensor_tensor(
                out=outb[:],
                in0=outb[:],
                in1=tb[:].rearrange("b (i j) -> b i j", j=WO),
                op=mybir.AluOpType.add,
            )
            nc.scalar.dma_start(out=out_d[:], in_=outb[:])

    nc.compile()
    return nc


def prep_inputs(x, z, b):
    """Host-side shard + layout prep. Returns per-core in_maps."""
    xb = np.asarray(x).astype(ml_dtypes.bfloat16)
    zb = np.asarray(z).astype(ml_dtypes.bfloat16)
    bb = np.asarray(b).astype(ml_dtypes.bfloat16)
    bias3 = bb.reshape(OO, B, C)
    in_maps = []
    for core in range(NCORES):
        b0 = core * BPC
        # xT[c, ch, b, p] = x[b, p//22, p%22, ch*128+c]
        xs = xb[b0 : b0 + BPC].reshape(BPC, P, C)
        xT = np.ascontiguousarray(
            xs.transpose(2, 0, 1).reshape(2, 128, BPC, P).transpose(1, 0, 2, 3)
        )
        # zT[c, b, ch, k, g] = z[b, DM*k + g//6, g%6, ch*128 + c]
        zs = zb[b0 : b0 + BPC].reshape(BPC, NK, G, C)
        zT = np.zeros((128, BPC, 2, NK, ZC), dtype=ml_dtypes.bfloat16)
        zT[..., :G] = (
            np.ascontiguousarray(zs.transpose(3, 0, 1, 2))
            .reshape(2, 128, BPC, NK, G)
            .transpose(1, 2, 0, 3, 4)
        )
        # bT[c, ch, b, o] = bias[o, b, ch*128 + c]
        bs = bias3[:, b0 : b0 + BPC, :]
        bT = np.ascontiguousarray(
            bs.transpose(2, 1, 0).reshape(2, 128, BPC, OO).transpose(1, 0, 2, 3)
        )
        in_maps.append({"xt": xT, "zt": zT, "bt": bT})
    return in_maps


_cache = {}


def _ensure_ntff_hook():
    """The axon NTFF profile hook normally lives in antenv.axon_hooks, which
    this image lacks; synthesize it from the boot shim's ctypes wrapper."""
    try:
        from antenv.axon_hooks import get_axon_ntff_profile_hook  # noqa: F401
        return True
    except ImportError:
        pass
    try:
        import sys, types
        from trn_agent_boot.trn_boot import _ntff_profile_via_ctypes

        so = os.environ.get("AXON_PJRT_SO", "/opt/axon/libaxon_pjrt.so")
        hook = _ntff_profile_via_ctypes(so)
        mod = types.ModuleType("antenv.axon_hooks")
        mod.get_axon_ntff_profile_hook = lambda: hook
        mod.set_axon_ntff_profile_hook = lambda h: None
        sys.modules["antenv.axon_hooks"] = mod
        import antenv

        antenv.axon_hooks = mod
        return True
    except Exception:
        return False


def kernel(x, z, b):
    from concourse.bass_utils import run_bass_kernel_spmd

    if "nc" not in _cache:
        _cache["nc"] = build_module()
    nc = _cache["nc"]
    in_maps = prep_inputs(x, z, b)
    trace = bool(int(os.environ.get("KERNEL_TRACE", "0") or 0))
    if trace:
        trace = _ensure_ntff_hook()
    res = run_bass_kernel_spmd(
        nc,
        in_maps,
        core_ids=list(range(NCORES)),
        trace=trace,
    )
    _cache["last_result"] = res
    out = np.concatenate([r["out"].reshape(BPC, HO, WO) for r in res.results], axis=0)
    return out[..., None].astype(np.float32)
